# revision 25
# baseline (speedup 1.0000x reference)
"""Self-contained Trainium2 Bass kernel for the 2-layer GAT (nn_GAT_6451040878848).

Sharding: nodes are permuted by in-degree and dealt round-robin to 8 cores;
core k owns a contiguous 12800-row octant of the permuted node table (tile 64
of each octant is a reserved all-pad tile) and aggregates every edge whose dst
is in its octant (dst-octant edge sharding -> no all-reduce). The only
collective is one bf16 AllGather of the layer-2 node table.

Edge phase: per 128-dst tile, edges are slot-major (slot c = c-th in-edge of
each dst; dst == partition), gathered from the node table with dma_gather
(int16 idx, elem_size 72/66 cols of the 128-col rows -> 144B/132B per edge).
The gather is descriptor-rate-bound (~5-6 ns/row on the Q7 descgen), so slot
count == descriptor count is the currency. The table is addressed through
THREE 65536-row windows at 34176-row spacing over a WRAPAROUND row space
(rows [0, 31488) are replicated after row 102400), giving every src row ~2
window choices with no poorly-covered end zones; per-dst routing solves a
small Hall-feasibility problem per tile, bringing pad slots to ~12% over the
intrinsic tiling bound (1895 vs 1688 slots/layer, vs 2189 for end-anchored
windows). Pad slots point at zero rows (alpha_s=-40 => w~0) inside
window-addressable pad tiles, so every index list tail is non-negative (the
ucode drops contiguous all-negative tails). Weights
w = exp(leaky_relu(alpha_s[src]+alpha_d[dst])) (no max-subtraction: logits
are O(1)); the scatter is an identity-lhsT matmul accumulating 4 slots per
Matmult into a [128, 4*vw] PSUM tile, folded with one tensor_reduce; the
softmax denominator rides along as extra rhs columns. alpha_d never
round-trips through DRAM: layer-1 alpha_d is copied to SBUF during the
(replicated) node phase, layer-2 alpha_d is written to SBUF by the
octant-local layer-2 node phase.
"""
import numpy as np
import ml_dtypes

import concourse.bacc as bacc
import concourse.bass as bass
import concourse.tile as tile
from concourse import mybir
from concourse.bass_utils import run_bass_kernel_spmd

P = 128
TROW = 128           # bf16 elements per node-table row (256B)
NEG_SLOPE = 0.2
F_IN = 128
H1, C1 = 8, 8
C2 = 64
DEG_QUANT = 1
NCORES = 8
BANK = 65536
MMG = 4              # slots per matmul group

# ---- fixed geometry (N = 100000 hardcoded) ----
N_NODES = 100000
PER_CORE = 12800             # 100 tiles; tile PAD_TILE is all-pad
N_TILES = PER_CORE // P      # 100
PAD_TILE = 64
J_REAL = 12544               # dealt node slots per core (before pad-tile insertion)
NPAD = PER_CORE * NCORES     # 102400 == rows_total
# Wraparound windows: rows [0, NWRAP) are replicated at [NPAD, NPAD+NWRAP) so
# three 64K-row windows tile the 102400-row circle with uniform ~2x coverage.
W_LO = [0, 34176, 68352]     # window starts (extended row space)
W_BASE = [lo + 32768 for lo in W_LO]
NWRAP = W_LO[2] + BANK - NPAD          # 31488
NPAD_EXT = NPAD + 32768                # 135168 (wrap alloc rounded up)
# pad rows (zero h, alpha=-40) inside each window, biased idx >= 0:
#   W0: octant-2 pad tile; W1: octant-5; W2: wrap copy of octant-0 pad tile
ROW_PADS = [2 * PER_CORE + PAD_TILE * P,          # 33792  (biased 1024)
            5 * PER_CORE + PAD_TILE * P,          # 72192  (biased 5248)
            NPAD + 0 * PER_CORE + PAD_TILE * P]   # 110592 (biased 9472)

bf16 = ml_dtypes.bfloat16


# ----------------------------------------------------------------------------
# Host-side graph preprocessing (integer/index work only)
# ----------------------------------------------------------------------------
def host_prep(edge_index: np.ndarray, n_nodes: int, n_cores: int = NCORES):
    assert n_nodes == N_NODES and n_cores == NCORES
    N = n_nodes
    loops = np.arange(N, dtype=np.int64)
    src = np.concatenate([edge_index[0].astype(np.int64), loops])
    dst = np.concatenate([edge_index[1].astype(np.int64), loops])

    deg = np.bincount(dst, minlength=N)
    degq = -(-deg // DEG_QUANT) * DEG_QUANT

    rank = np.argsort(degq, kind="stable")
    node_rank = np.empty(N, np.int64)
    node_rank[rank] = np.arange(N)
    core_of = node_rank % n_cores
    j_of = node_rank // n_cores              # < 12500 <= J_REAL
    l_of = j_of + np.where(j_of >= PAD_TILE * P, P, 0)
    pos = core_of * PER_CORE + l_of          # == table row
    node_of_pos = np.full(NPAD, -1, dtype=np.int64)
    node_of_pos[pos] = np.arange(N)

    e_core = core_of[dst]
    e_tile = l_of[dst] // P
    e_part = l_of[dst] % P
    r = pos[src]

    # window membership (wraparound): W0=[0,64K), W1=[34176,99712),
    # W2=[68352,133888) with rows [0,31488) aliased at +NPAD.
    in0 = r < W_LO[0] + BANK
    in1 = (r >= W_LO[1]) & (r < W_LO[1] + BANK)
    in2 = (r >= W_LO[2]) | (r < NWRAP)
    # classes: 0:{0} 1:{1} 2:{2} 3:{0,1} 4:{1,2} 5:{0,2}
    cls = np.where(in0 & in1, 3,
          np.where(in1 & in2, 4,
          np.where(in0 & in2, 5,
          np.where(in0, 0, np.where(in1, 1, 2)))))

    NCLS = 6
    cnt = np.zeros((NCLS, n_cores, N_TILES, P), np.int32)
    for c in range(NCLS):
        m = cls == c
        np.add.at(cnt[c], (e_core[m], e_tile[m], e_part[m]), 1)

    # ---- per-tile (D0, D1, D2) optimization (Hall conditions over buckets) ----
    DA = np.zeros(N_TILES, np.int64)
    DB = np.zeros(N_TILES, np.int64)
    DC = np.zeros(N_TILES, np.int64)
    SPAN = 12
    for t in range(N_TILES):
        n = [cnt[c, :, t, :].ravel().astype(np.int64) for c in range(NCLS)]
        tot = sum(n)
        if tot.max() == 0:
            continue
        lb0 = int(n[0].max()); lb1 = int(n[1].max()); lb2 = int(n[2].max())
        pair01 = int((n[0] + n[1] + n[3]).max())
        d0s = np.arange(lb0, lb0 + SPAN)[:, None, None]
        d1s = np.arange(lb1, lb1 + SPAN)[None, :, None]
        shape = (SPAN, SPAN, len(tot))
        req12 = np.broadcast_to((n[1] + n[2] + n[4])[None, None, :] - d1s, shape)
        req02 = np.broadcast_to((n[0] + n[2] + n[5])[None, None, :] - d0s, shape)
        reqT = np.broadcast_to(tot[None, None, :] - d0s - d1s, shape)
        d2 = np.maximum.reduce([
            np.broadcast_to(np.int64(lb2), shape),
            req12, req02, reqT]).max(axis=2)
        total = d0s[:, :, 0] + d1s[:, :, 0] + np.maximum(d2, 0)
        total = np.where(d0s[:, :, 0] + d1s[:, :, 0] >= pair01, total, 10 ** 9)
        i, j = np.unravel_index(np.argmin(total), total.shape)
        DA[t] = lb0 + i; DB[t] = lb1 + j
        DC[t] = int(max(np.maximum(d2, 0)[i, j], 0))

    # ---- per-dst routing honoring (D0, D1, D2) ----
    dstkey = (e_core * N_TILES + e_tile) * P + e_part
    order = np.lexsort((cls, dstkey))
    sk = dstkey[order]; sc = cls[order]
    grp_start = np.concatenate([[0], np.nonzero(np.diff(sk))[0] + 1, [len(sk)]])
    ebank = np.empty(len(order), np.int8)
    slot = np.empty(len(order), np.int64)
    tile_of_key = (np.arange(n_cores * N_TILES * P) // P) % N_TILES
    for gi in range(len(grp_start) - 1):
        s0, s1 = grp_start[gi], grp_start[gi + 1]
        key = sk[s0]; t = tile_of_key[key]
        d0, d1, d2 = int(DA[t]), int(DB[t]), int(DC[t])
        cl = sc[s0:s1]
        c = [int((cl == cc).sum()) for cc in range(NCLS)]
        sp0 = d0 - c[0]; sp1 = d1 - c[1]; sp2 = d2 - c[2]
        assert sp0 >= 0 and sp1 >= 0 and sp2 >= 0, (t, d0, d1, d2, c)
        # route pair classes: x3 of cls3 -> W0 (rest W1); x4 of cls4 -> W1
        # (rest W2); x5 of cls5 -> W2 (rest W0)
        sol = None
        for x3 in range(min(c[3], sp0), -1, -1):
            s1r = sp1 - (c[3] - x3)
            if s1r < 0:
                continue
            x4 = min(c[4], s1r)
            s2r = sp2 - (c[4] - x4)
            if s2r < 0:
                continue
            x5 = min(c[5], s2r)
            if c[5] - x5 <= sp0 - x3:
                sol = (x3, x4, x5)
                break
        assert sol is not None, (t, d0, d1, d2, c)
        x3, x4, x5 = sol
        bk = np.empty(s1 - s0, np.int8)
        u3 = u4 = u5 = 0
        for i, cc in enumerate(cl):
            if cc == 0:
                bk[i] = 0
            elif cc == 1:
                bk[i] = 1
            elif cc == 2:
                bk[i] = 2
            elif cc == 3:
                bk[i] = 0 if u3 < x3 else 1
                u3 += 1
            elif cc == 4:
                bk[i] = 1 if u4 < x4 else 2
                u4 += 1
            else:
                bk[i] = 2 if u5 < x5 else 0
                u5 += 1
        sl = np.empty(s1 - s0, np.int64)
        ca = cb = cc2 = 0
        for i in range(s1 - s0):
            if bk[i] == 0:
                sl[i] = ca; ca += 1
            elif bk[i] == 1:
                sl[i] = cb; cb += 1
            else:
                sl[i] = cc2; cc2 += 1
        ebank[s0:s1] = bk
        slot[s0:s1] = sl

    # ---- trailing-negative terminators ----
    # The ucode drops a contiguous all-negative tail of each gather's idx
    # list, so the LAST list position (slot d-1, part 127) of each bank
    # segment must be non-negative. If partition 127's segment is full of
    # negative-idx reals for some core, widen that bank by one all-pad slot.
    bases = np.array(W_BASE, np.int64)
    padrow = np.array(ROW_PADS, np.int64)
    # extended row per (edge, window): wrap rows alias at +NPAD for W2
    r_ext = r.copy()
    so_core = e_core[order]; so_row = r[order]; so_rext = r_ext[order]
    so_tile = e_tile[order]; so_part = e_part[order]
    so_bank = ebank; so_slot = slot
    Ds = np.stack([DA, DB, DC], axis=1)           # [T, 3]
    m127 = so_part == 127
    for t in range(N_TILES):
        for b in range(3):
            d = int(Ds[t, b])
            if d == 0:
                continue
            m = m127 & (so_tile == t) & (so_bank == b)
            if not m.any():
                continue
            rows = so_row[m]
            biased = np.where((b == 2) & (rows < NWRAP), rows + NPAD, rows) - bases[b]
            cores = so_core[m]
            for k in np.unique(cores):
                mk = cores == k
                if mk.sum() == d and (biased[mk] < 0).all():
                    Ds[t, b] = d + 1
                    break
    DA, DB, DC = Ds[:, 0], Ds[:, 1], Ds[:, 2]

    # ---- idx16 arrays (block-bank-major: per 4-tile block [A t0..t3|B ...|C ...]) ----
    TB = 4
    n_blocks = N_TILES // TB
    Dsum = DA + DB + DC
    # per-tile slot tables (bank-segment layout [A|B|C] per tile)
    tabs = []
    for t in range(N_TILES):
        ds = int(Dsum[t])
        if ds == 0:
            tabs.append(None)
            continue
        tab = np.empty((n_cores, ds, P), np.int64)
        for b in range(3):
            lo = (0, DA[t], DA[t] + DB[t])[b]
            d = (DA[t], DB[t], DC[t])[b]
            if d:
                tab[:, lo:lo + d, :] = padrow[b] - bases[b]
        m = so_tile == t
        seg_lo = np.array([0, DA[t], DA[t] + DB[t]], np.int64)
        abs_slot = seg_lo[so_bank[m]] + so_slot[m]
        rows_m = so_row[m]; bk_m = so_bank[m]
        rows_ext = np.where((bk_m == 2) & (rows_m < NWRAP), rows_m + NPAD, rows_m)
        biased = rows_ext - bases[bk_m]
        tab[so_core[m], abs_slot, so_part[m]] = biased
        assert tab.min() >= -32768 and tab.max() <= 32767
        for b in range(3):
            lo = (0, DA[t], DA[t] + DB[t])[b]
            d = int((DA[t], DB[t], DC[t])[b])
            if d == 0:
                continue
            for k in range(n_cores):
                col = tab[k, lo:lo + d, 127]
                if col[d - 1] < 0:
                    nn = np.nonzero(col >= 0)[0]
                    assert len(nn), (t, b, k, "all-negative p127 segment")
                    j = nn[-1]
                    col[d - 1], col[j] = col[j], col[d - 1]
        tabs.append(tab)

    idx_cols = int(Dsum.sum()) * 8
    idx16 = np.zeros((n_cores, 16, max(idx_cols, 1)), np.int16)
    blk_off = []           # column offset of each block
    cols = 0
    for blk in range(n_blocks):
        blk_off.append(cols)
        for b in range(3):
            for ti in range(TB):
                t = blk * TB + ti
                lo = (0, DA[t], DA[t] + DB[t])[b]
                d = int((DA[t], DB[t], DC[t])[b])
                if d == 0 or tabs[t] is None:
                    continue
                seg = tabs[t][:, lo:lo + d, :]
                idx16[:, :, cols:cols + d * 8] = (
                    seg.reshape(n_cores, -1, 16).transpose(0, 2, 1).astype(np.int16))
                cols += d * 8
    assert cols == idx_cols
    idx16 = np.tile(idx16, (1, 8, 1))

    meta = dict(
        N=N, Npad=NPAD, n_cores=n_cores, per_core=PER_CORE, n_tiles=N_TILES,
        DA=[int(v) for v in DA], DB=[int(v) for v in DB], DC=[int(v) for v in DC],
        TB=TB, blk_off=blk_off, idx_cols=int(max(idx_cols, 1)),
    )
    return meta, idx16, node_of_pos


def _dma_gather_narrow(eng, out_ap, in_ap, idxs_ap, num_idxs, elem_size,
                       elem_step, queue_num, single_packet=False):
    """dma_gather with elem_size_bytes not a multiple of 256.

    The bass wrapper asserts elem_size_bytes % 256 == 0 unconditionally, but
    on-device (decode/dma_gather.hpp) that restriction only applies to
    transpose mode; the non-transpose ucode packs arbitrary elem sizes.
    Mirrors the tail of BassGpSimd.dma_gather for the non-transpose,
    DRAM-source case.
    """
    from concourse import ap_utils
    assert idxs_ap.tensor.dtype == mybir.dt.int16
    assert ap_utils.ap_is_contiguous(out_ap.ap[1:])
    assert ap_utils.ap_is_contiguous(idxs_ap.ap[1:])
    assert in_ap.ap[0][0] == elem_step
    stride_bytes = elem_step * mybir.dt.size(in_ap.tensor.dtype)
    assert stride_bytes % 256 == 0
    stride_bytes_256 = stride_bytes // 256
    _in_ap = eng.lower_ap_dma(in_ap, for_custom_bir_dma=True)
    _idxs_ap = eng.lower_ap(idxs_ap)
    _out_ap = eng.lower_ap(out_ap)
    return eng.add_instruction(
        mybir.InstDMAGatherAnt(
            name=eng.bass.get_next_instruction_name(),
            ins=[*_in_ap, _idxs_ap, eng.lower_val_access(eng.to_reg(num_idxs))],
            outs=[_out_ap],
            transpose=False,
            num_idxs=num_idxs,
            elem_size=elem_size,
            stride_bytes_256=stride_bytes_256,
            gen_mode=0,
            single_packet=single_packet,
            queue_num=queue_num,
            sbuf_tokens_per_rank=0,
            sbuf_free_dim_per_rank=0,
            sbuf_free_dim_pad_per_rank=0,
            sbuf_byte_offset=0,
        )
    )


# ----------------------------------------------------------------------------
# Device kernel
# ----------------------------------------------------------------------------
def build_kernel(meta):
    import os
    Npad = meta["Npad"]; n_cores = meta["n_cores"]; per_core = meta["per_core"]
    n_tiles = meta["n_tiles"]
    DA, DB, DC = meta["DA"], meta["DB"], meta["DC"]
    idx_cols = meta["idx_cols"]
    n_groups_per_oct = per_core // (4 * P)        # 25

    TB = meta["TB"]; blk_off = meta["blk_off"]
    n_blocks = n_tiles // TB
    nc = bacc.Bacc("TRN2", target_bir_lowering=False, debug=False,
                   num_devices=n_cores, num_swdge_queues=4)
    f32, b16, i16 = mybir.dt.float32, mybir.dt.bfloat16, mybir.dt.int16
    AF = mybir.ActivationFunctionType
    OP = mybir.AluOpType

    xT = nc.dram_tensor("xT", [F_IN, Npad], b16, kind="ExternalInput").ap()
    W1 = nc.dram_tensor("W1", [F_IN, 64], f32, kind="ExternalInput").ap()
    W1T = nc.dram_tensor("W1T", [64, F_IN], f32, kind="ExternalInput").ap()
    A1 = nc.dram_tensor("A1", [64, 16], f32, kind="ExternalInput").ap()
    W2 = nc.dram_tensor("W2", [64, C2], f32, kind="ExternalInput").ap()
    W2T = nc.dram_tensor("W2T", [C2, 64], f32, kind="ExternalInput").ap()
    A2 = nc.dram_tensor("A2", [C2, 2], f32, kind="ExternalInput").ap()
    B1 = nc.dram_tensor("B1", [1, 64], f32, kind="ExternalInput").ap()
    B2 = nc.dram_tensor("B2", [1, C2], f32, kind="ExternalInput").ap()
    IDX = nc.dram_tensor("IDX", [P, idx_cols], i16, kind="ExternalInput").ap()
    OUT = nc.dram_tensor("OUT", [per_core, C2], f32, kind="ExternalOutput").ap()

    _gb = int(os.environ.get("E_GBUFS", "4"))
    with tile.TileContext(nc) as tc:
        with tc.tile_pool(name="dram", bufs=1, space="DRAM") as dram, \
             tc.tile_pool(name="consts", bufs=1) as cp, \
             tc.tile_pool(name="stg", bufs=2 if _gb >= 4 else 3) as nsp, \
             tc.tile_pool(name="xtp", bufs=2 if _gb >= 4 else 3) as xtp, \
             tc.tile_pool(name="gpl", bufs=_gb) as gp, \
             tc.tile_pool(name="vwp", bufs=2 if _gb >= 4 else 3) as vwp, \
             tc.tile_pool(name="stat", bufs=2 if _gb >= 4 else 4) as sp, \
             tc.tile_pool(name="pacc", bufs=2, space="PSUM") as pacc, \
             tc.tile_pool(name="pnode", bufs=2, space="PSUM") as pnode, \
             tc.tile_pool(name="ptr", bufs=2, space="PSUM") as ptr, \
             tc.tile_pool(name="pl2", bufs=2, space="PSUM") as pl2:

            table1 = dram.tile([NPAD_EXT, TROW], b16)
            table2 = dram.tile([NPAD_EXT, TROW], b16)
            h2loc = dram.tile([per_core, TROW], b16)

            NW_CH = 6
            ch_rows = NWRAP // NW_CH           # 5248 rows per chunk
            assert ch_rows * NW_CH == NWRAP and ch_rows % P == 0

            def _wrap_copy(table):
                for ci in range(NW_CH):
                    stg = nsp.tile([P, (ch_rows // P) * TROW], b16, tag="wrapc")
                    stg3 = stg[:].rearrange("p (a r) -> p a r", r=TROW)
                    lo = ci * ch_rows
                    nc.sync.dma_start(
                        out=stg3,
                        in_=table[lo:lo + ch_rows, :]
                            .rearrange("(a p) r -> p a r", p=P))
                    nc.sync.dma_start(
                        out=table[NPAD + lo:NPAD + lo + ch_rows, :]
                            .rearrange("(a p) r -> p a r", p=P),
                        in_=stg3)

            # ------------- constants -------------
            ident = cp.tile([P, P], b16)
            nc.gpsimd.memset(ident[:], 0.0)
            iota_i = cp.tile([P, 1], mybir.dt.int32)
            nc.gpsimd.iota(iota_i[:], pattern=[[0, 1]], base=0, channel_multiplier=1)
            iota_f = cp.tile([P, 1], f32)
            nc.vector.tensor_copy(out=iota_f[:], in_=iota_i[:])
            iotar_i = cp.tile([P, P], mybir.dt.int32)
            nc.gpsimd.iota(iotar_i[:], pattern=[[1, P]], base=0, channel_multiplier=0)
            iotar_f = cp.tile([P, P], f32)
            nc.vector.tensor_copy(out=iotar_f[:], in_=iotar_i[:])
            nc.vector.tensor_scalar(out=ident[:], in0=iotar_f[:], scalar1=iota_f[:],
                                    scalar2=None, op0=OP.is_equal)

            w1f = cp.tile([P, 64], f32)
            nc.sync.dma_start(out=w1f[:], in_=W1)
            w1t = cp.tile([64, P], f32)
            nc.sync.dma_start(out=w1t[:], in_=W1T)
            a1t = cp.tile([64, 16], f32)
            nc.sync.dma_start(out=a1t[:], in_=A1)
            w2f = cp.tile([64, 64], f32)
            nc.sync.dma_start(out=w2f[:], in_=W2)
            w2t = cp.tile([64, 64], f32)
            nc.sync.dma_start(out=w2t[:], in_=W2T)
            a2t = cp.tile([64, 2], f32)
            nc.sync.dma_start(out=a2t[:], in_=A2)
            b1r = cp.tile([1, 64], f32)
            nc.sync.dma_start(out=b1r[:], in_=B1)
            b1b = cp.tile([P, 64], f32)
            nc.gpsimd.partition_broadcast(b1b[:], b1r[:])
            b2r = cp.tile([1, 64], f32)
            nc.sync.dma_start(out=b2r[:], in_=B2)
            b2b = cp.tile([P, 64], f32)
            nc.gpsimd.partition_broadcast(b2b[:], b2r[:])

            wext1 = cp.tile([P, 80], b16)
            ws_ps = pnode.tile([P, 352], f32, space="PSUM", tag="np")
            nc.tensor.matmul(out=ws_ps[:, 0:16], lhsT=w1t[:], rhs=a1t[:], start=True, stop=True)
            nc.vector.tensor_copy(out=wext1[:, 0:64], in_=w1f[:])
            nc.vector.tensor_copy(out=wext1[:, 64:80], in_=ws_ps[:, 0:16])

            w2ext = cp.tile([64, 66], b16)
            ws2_ps = pnode.tile([P, 352], f32, space="PSUM", tag="np")
            nc.tensor.matmul(out=ws2_ps[:64, 0:2], lhsT=w2t[:], rhs=a2t[:], start=True, stop=True)
            nc.vector.tensor_copy(out=w2ext[:, 0:64], in_=w2f[:])
            nc.vector.tensor_copy(out=w2ext[:, 64:66], in_=ws2_ps[:64, 0:2])

            # pad-row template: h = 0, alpha = -40
            padt = cp.tile([P, 16], b16)
            nc.gpsimd.memset(padt[:], -40.0)
            padt2 = cp.tile([P, 66], b16)
            nc.gpsimd.memset(padt2[:], 0.0)
            nc.gpsimd.memset(padt2[:, 64:66], -40.0)

            # alpha_d SBUF tables
            ad1_all = cp.tile([P, n_cores * n_tiles * 8], b16)
            ad2_all = cp.tile([P, n_tiles], f32)

            pid = nc.partition_id()

            # ------------- L1 node phase (replicated) -------------
            # Packed layout: matmul i uses lhsT = xt[:, i::4] so output
            # partition p' holds node 4p'+i; the 512-row table store is then
            # per-partition contiguous (128 descs x 1KB instead of 512 x 160B).
            for _nrep in range(int(os.environ.get("KREP_NODE", "1"))):
              for k in range(n_cores):
                for g0 in range(0, n_groups_per_oct, 2):
                    gs = [g for g in range(g0, g0 + 2) if g < n_groups_per_oct]
                    nb = len(gs)
                    base = k * per_core + g0 * 4 * P
                    xt2 = xtp.tile([P, nb * 4 * P], b16, tag="xt")
                    nc.sync.dma_start(out=xt2[:], in_=xT[:, base:base + nb * 4 * P])
                    stage = nsp.tile([P, nb * 4 * TROW], b16, tag="stage")
                    for bi, g in enumerate(gs):
                        xt = xt2[:, bi * 4 * P:(bi + 1) * 4 * P]
                        xtv = xt.rearrange("f (s i) -> f s i", i=4)
                        ps = pnode.tile([P, 352], f32, space="PSUM", tag="np")
                        for i in range(4):
                            nc.tensor.matmul(out=ps[:, i * 72:(i + 1) * 72],
                                             lhsT=xtv[:, :, i],
                                             rhs=wext1[:, 0:72], start=True, stop=True)
                        # alpha_d (node-per-partition) in cols 288:320
                        for s in range(4):
                            nc.tensor.matmul(out=ps[:, 288 + s * 8:288 + (s + 1) * 8],
                                             lhsT=xt[:, s * P:(s + 1) * P],
                                             rhs=wext1[:, 72:80], start=True, stop=True)
                        nc.scalar.activation(
                            out=stage[:, bi * 4 * TROW:(bi + 1) * 4 * TROW]
                                .rearrange("p (i r) -> p i r", r=TROW)[:, :, 0:72],
                            in_=ps[:, 0:288].rearrange("p (i r) -> p i r", r=72),
                            func=AF.Copy)
                        nc.vector.tensor_copy(
                            out=ad1_all[:, (k * n_tiles + g * 4) * 8:(k * n_tiles + g * 4 + 4) * 8],
                            in_=ps[:, 288:320])
                    # one store for the pair: row = base + b*512 + p*4 + i
                    nc.sync.dma_start(
                        out=table1[base:base + nb * 4 * P, :]
                            .rearrange("(b p i) r -> p b (i r)", b=nb, p=P),
                        in_=stage[:].rearrange("p (b x) -> p b x", b=nb))
            # patch pad tiles: alpha_s (cols 64:72) = -40  (h already 0)
            for k in range(n_cores):
                r0 = k * per_core + PAD_TILE * P
                nc.sync.dma_start(
                    out=table1[r0:r0 + P, 64:72].rearrange("(o p) r -> p (o r)", p=P),
                    in_=padt[:, 0:8])
            # wraparound replica: rows [0, NWRAP) -> [NPAD, NPAD+NWRAP)
            # (staged through SBUF: same-tile DRAM->DRAM copies are not
            # reliably dependency-ordered)
            _wrap_copy(table1)

            # ------------- edge phase -------------
            # block geometry: per 4-tile block, bank-major concatenated segments
            blocks = []
            for blk in range(n_blocks):
                segs = []          # (bank, t, blk_slot_off, d)
                calls = []         # (bank, call_slot_off, call_len)
                goff = 0
                for b in range(3):
                    call_lo = goff
                    for ti in range(TB):
                        t = blk * TB + ti
                        d = (DA[t], DB[t], DC[t])[b]
                        if d:
                            segs.append((b, t, goff, d))
                            goff += d
                    if goff > call_lo:
                        calls.append((b, call_lo, goff - call_lo))
                blocks.append((segs, calls, goff))
            DsblkMax = max(g for _, _, g in blocks)

            idxall = cp.tile([P, idx_cols], i16)
            nc.sync.dma_start(out=idxall[:], in_=IDX)

            g_shared = [None]

            def edge_phase(table, layer, emit):
                nogather = os.environ.get("E_NOGATHER") == "1"
                nocompute = os.environ.get("E_NOCOMPUTE") == "1"
                heads = H1 if layer == 1 else 1
                vw = 72 if layer == 1 else 65
                # gathered row width (elements): h + alpha columns only
                GW = (72 if layer == 1 else 66) if os.environ.get("E_ELEM", "1") == "1" \
                    else TROW
                if nogather and g_shared[0] is None:
                    gsh = cp.tile([P, DsblkMax * TROW], b16)
                    nc.gpsimd.memset(gsh[:], 0.125)
                    g_shared[0] = gsh
                # gather base address = biased-idx zero point (idx can be
                # negative, reading down to W_LO[w])
                in_ap = (table[W_BASE[0]:W_BASE[0] + 32768, :],
                         table[W_BASE[1]:W_BASE[1] + 32768, :],
                         table[W_BASE[2]:W_BASE[2] + 32768, :])
                hotidx = os.environ.get("E_HOTIDX") == "1"
                qctr = 0
                for blk in range(n_blocks):
                    segs, calls, dsblk = blocks[blk if not hotidx else 0]
                    if dsblk == 0:
                        continue
                    G = g_shared[0] if nogather else gp.tile([P, dsblk * GW], b16, tag="G")
                    if not nogather:
                        qmode = os.environ.get("E_QMODE", "0")
                        for b, call_lo, dlen in calls:
                            c0 = blk_off[blk if not hotidx else 0] + call_lo * 8
                            if qmode == "2" and dlen > 1:
                                h1_ = dlen // 2
                                for (clo, dl2) in ((call_lo, h1_), (call_lo + h1_, dlen - h1_)):
                                    c2 = blk_off[blk if not hotidx else 0] + clo * 8
                                    _dma_gather_narrow(
                                        nc.gpsimd,
                                        out_ap=G[:, clo * GW:(clo + dl2) * GW]
                                            .rearrange("p (s r) -> p s r", r=GW),
                                        in_ap=in_ap[b], idxs_ap=idxall[:, c2:c2 + dl2 * 8],
                                        num_idxs=dl2 * P,
                                        elem_size=GW, elem_step=TROW,
                                        queue_num=qctr % 4, single_packet=False)
                                    qctr += 1
                                continue
                            q = (blk % 4) if qmode == "1" else (qctr % 4)
                            _dma_gather_narrow(
                                nc.gpsimd,
                                out_ap=G[:, call_lo * GW:(call_lo + dlen) * GW]
                                    .rearrange("p (s r) -> p s r", r=GW),
                                in_ap=in_ap[b], idxs_ap=idxall[:, c0:c0 + dlen * 8],
                                num_idxs=dlen * P, elem_size=GW,
                                elem_step=TROW,
                                queue_num=q, single_packet=os.environ.get("E_SP","0")=="1")
                            qctr += 1
                    if nocompute:
                        continue

                    Gv = G[:, 0:dsblk * GW].rearrange("p (s r) -> p s r", r=GW)
                    if layer == 1:
                        ad_off = pid * (n_tiles * 8) + blk * TB * 8
                        ad_blk = sp.tile([P, TB * 8], b16, tag="adt")
                        nc.vector.tensor_copy(out=ad_blk[:], in_=ad1_all[:, bass.ds(ad_off, TB * 8)])
                    wb_all = sp.tile([P, dsblk * heads], b16, tag="wb")
                    e_t = sp.tile([P, dsblk * heads], b16, tag="et")
                    if layer == 1:
                        for b, t, off, d in segs:
                            ti = t - blk * TB
                            adv = ad_blk[:, ti * 8:(ti + 1) * 8] \
                                .unsqueeze(1).broadcast_to([P, d, 8])
                            nc.vector.tensor_tensor(
                                out=e_t[:, off * 8:(off + d) * 8]
                                    .rearrange("p (s h) -> p s h", h=8),
                                in0=Gv[:, off:off + d, 64:72], in1=adv, op=OP.add)
                    else:
                        for b, t, off, d in segs:
                            nc.vector.tensor_scalar(
                                out=e_t[:, off:off + d],
                                in0=Gv[:, off:off + d, 64:65]
                                    .rearrange("p s one -> p (s one)"),
                                scalar1=ad2_all[:, t:t + 1], scalar2=None, op0=OP.add)
                    with nc.allow_low_precision(reason="logits O(1), bf16 weights"):
                        nc.scalar.activation(out=e_t[:], in_=e_t[:],
                                             func=AF.Prelu, alpha=NEG_SLOPE)
                        nc.scalar.activation(out=wb_all[:], in_=e_t[:], func=AF.Exp)

                    # per-tile V staging + scatter matmuls
                    for ti in range(TB):
                        t = blk * TB + ti
                        ds = DA[t] + DB[t] + DC[t]
                        if ds == 0:
                            continue
                        tsegs = [(b, off, d) for (b, t2, off, d) in segs if t2 == t]
                        nmm = -(-ds // MMG)
                        ds_pad = nmm * MMG
                        Vw = vwp.tile([P, ds_pad * vw], b16, tag="vw")
                        if ds_pad > ds:
                            nc.gpsimd.memset(Vw[:, ds * vw:ds_pad * vw], 0.0)
                        Vv = Vw[:].rearrange("p (s c) -> p s c", c=vw)[:, 0:ds, :]
                        tloc = 0
                        for b, off, d in tsegs:
                            if layer == 1:
                                wbv = wb_all[:].rearrange("p (s h) -> p s h", h=8) \
                                    [:, off:off + d, :].unsqueeze(3) \
                                    .broadcast_to([P, d, 8, 8])
                                nc.vector.tensor_tensor(
                                    out=Vv[:, tloc:tloc + d, 0:64]
                                        .rearrange("p s (h c) -> p s h c", c=8),
                                    in0=Gv[:, off:off + d, 0:64]
                                        .rearrange("p s (h c) -> p s h c", c=8),
                                    in1=wbv, op=OP.mult)
                                nc.vector.tensor_copy(
                                    out=Vv[:, tloc:tloc + d, 64:72],
                                    in_=wb_all[:].rearrange("p (s h) -> p s h", h=8)
                                        [:, off:off + d, :])
                            else:
                                wbv = wb_all[:, off:off + d].unsqueeze(2) \
                                    .broadcast_to([P, d, 64])
                                nc.vector.tensor_tensor(
                                    out=Vv[:, tloc:tloc + d, 0:64],
                                    in0=Gv[:, off:off + d, 0:64], in1=wbv, op=OP.mult)
                                nc.vector.tensor_copy(
                                    out=Vv[:, tloc:tloc + d, 64:65],
                                    in_=wb_all[:, off:off + d].unsqueeze(2))
                            tloc += d

                        acc = pacc.tile([P, MMG * 72], f32, space="PSUM", tag="acc")
                        for i in range(nmm):
                            nc.tensor.matmul(out=acc[:, 0:MMG * vw], lhsT=ident[:],
                                             rhs=Vw[:, i * MMG * vw:(i + 1) * MMG * vw],
                                             start=(i == 0), stop=(i == nmm - 1))
                        emit(t, acc, MMG, vw)

            # ------------- L1 -------------
            out1 = cp.tile([P, n_tiles * 72], b16)

            def emit1(t, acc, ngrp, vw):
                with nc.allow_low_precision(reason="4-way fold to bf16 staging"):
                    nc.vector.tensor_reduce(
                        out=out1[:, t * 72:(t + 1) * 72],
                        in_=acc[:, 0:ngrp * vw].rearrange("p (g c) -> p c g", c=vw),
                        op=OP.add, axis=mybir.AxisListType.X)

            if os.environ.get("SKIP_E1") == "1":
                nc.gpsimd.memset(out1[:], 1.0)
            else:
                for _erep in range(int(os.environ.get("KREP_E1", "1"))):
                    edge_phase(table1, 1, emit1)
                if os.environ.get("E_NOCOMPUTE") == "1":
                    nc.gpsimd.memset(out1[:], 1.0)

            # ------------- L2 node phase (octant-local) -------------
            for t in range(n_tiles):
                if t == PAD_TILE:
                    nc.sync.dma_start(out=h2loc[t * P:(t + 1) * P, 0:66], in_=padt2[:])
                    continue
                if DA[t] + DB[t] + DC[t] == 0:
                    continue
                den = sp.tile([P, 8], f32, tag="den")
                nc.vector.tensor_copy(out=den[:], in_=out1[:, t * 72 + 64:t * 72 + 72])
                rec = sp.tile([P, 8], f32, tag="rec")
                nc.vector.reciprocal(rec[:], den[:])
                recb = sp.tile([P, 8], b16, tag="recb")
                nc.vector.tensor_copy(out=recb[:], in_=rec[:])
                h1f = sp.tile([P, 64], f32, tag="h1f")
                nc.vector.tensor_tensor(
                    out=h1f[:].rearrange("p (h c) -> p h c", c=8),
                    in0=out1[:, t * 72:t * 72 + 64].rearrange("p (h c) -> p h c", c=8),
                    in1=recb[:].unsqueeze(2).broadcast_to([P, 8, 8]), op=OP.mult)
                nc.vector.tensor_tensor(out=h1f[:], in0=h1f[:], in1=b1b[:], op=OP.add)
                # ELU: out = max(x,0) + exp(min(x,0)) - 1
                xm = sp.tile([P, 64], f32, tag="xm")
                nc.vector.tensor_scalar(out=xm[:], in0=h1f[:], scalar1=0.0,
                                        scalar2=None, op0=OP.min)
                xe = sp.tile([P, 64], f32, tag="xe")
                nc.scalar.activation(out=xe[:], in_=xm[:], func=AF.Exp)
                xp = sp.tile([P, 64], b16, tag="xp")
                nc.vector.tensor_scalar(out=xp[:], in0=h1f[:], scalar1=0.0,
                                        scalar2=None, op0=OP.max)
                h1e = sp.tile([P, 64], b16, tag="h1e")
                nc.vector.tensor_scalar(out=h1e[:], in0=xe[:], scalar1=-1.0,
                                        scalar2=None, op0=OP.add, accum_out=None)
                nc.vector.tensor_tensor(out=h1e[:], in0=h1e[:], in1=xp[:], op=OP.add)
                trp = ptr.tile([P, P], b16, space="PSUM", tag="tr")
                nc.tensor.transpose(out=trp[:64, :], in_=h1e[:], identity=ident[:])
                h1t = sp.tile([64, P], b16, tag="h1t")
                nc.scalar.activation(out=h1t[:], in_=trp[:64, :], func=AF.Copy)
                ps2 = pl2.tile([P, 80], f32, space="PSUM", tag="l2")
                nc.tensor.matmul(out=ps2[:, 0:66], lhsT=h1t[:], rhs=w2ext[:],
                                 start=True, stop=True)
                st2 = nsp.tile([P, 65], b16, tag="st2")
                nc.scalar.activation(out=st2[:], in_=ps2[:, 0:65], func=AF.Copy)
                nc.sync.dma_start(out=h2loc[t * P:(t + 1) * P, 0:65], in_=st2[:])
                nc.vector.tensor_copy(out=ad2_all[:, t:t + 1], in_=ps2[:, 65:66])

            # ------------- AllGather h2 octants -> table2 -------------
            if os.environ.get("SKIP_CC") == "1":
                for k in range(n_cores):
                    nc.sync.dma_start(
                        out=table2[k * per_core:(k + 1) * per_core, :],
                        in_=h2loc[:])
            else:
                nc.gpsimd.collective_compute(
                    "AllGather", mybir.AluOpType.bypass,
                    replica_groups=[list(range(n_cores))],
                    ins=[h2loc[:].opt()],
                    outs=[table2[0:Npad, :].opt()],
                )
            _wrap_copy(table2)
            # completion fence: L2 gathers must not start until the AllGather
            # payload and wrap replica are fully landed (cross-core visibility)
            tc.strict_bb_all_engine_barrier()

            # ------------- L2 -------------
            def emit2(t, acc, ngrp, vw):
                o2 = sp.tile([P, 65], f32, tag="o2")
                nc.vector.tensor_reduce(
                    out=o2[:],
                    in_=acc[:, 0:ngrp * vw].rearrange("p (g c) -> p c g", c=vw),
                    op=OP.add, axis=mybir.AxisListType.X)
                rec2 = sp.tile([P, 1], f32, tag="rec2")
                nc.vector.reciprocal(rec2[:], o2[:, 64:65])
                o2n = sp.tile([P, 64], f32, tag="o2n")
                nc.vector.tensor_scalar(out=o2n[:], in0=o2[:, 0:64], scalar1=rec2[:],
                                        scalar2=None, op0=OP.mult)
                nc.vector.tensor_tensor(out=o2n[:], in0=o2n[:], in1=b2b[:], op=OP.add)
                m = sp.tile([P, 1], f32, tag="m")
                nc.vector.tensor_reduce(out=m[:], in_=o2n[:], op=OP.max,
                                        axis=mybir.AxisListType.X)
                negm = sp.tile([P, 1], f32, tag="negm")
                nc.vector.tensor_scalar(out=negm[:], in0=m[:], scalar1=-1.0,
                                        scalar2=None, op0=OP.mult)
                scr = sp.tile([P, 64], f32, tag="scr")
                sume = sp.tile([P, 1], f32, tag="sume")
                nc.scalar.activation(out=scr[:], in_=o2n[:], func=AF.Exp,
                                     bias=negm[:], accum_out=sume[:])
                lns = sp.tile([P, 1], f32, tag="lns")
                nc.scalar.activation(out=lns[:], in_=sume[:], func=AF.Ln)
                res = sp.tile([P, 64], f32, tag="res")
                nc.vector.tensor_scalar(out=res[:], in0=o2n[:], scalar1=m[:],
                                        scalar2=lns[:], op0=OP.subtract,
                                        op1=OP.subtract)
                nc.sync.dma_start(out=OUT[t * P:(t + 1) * P, :], in_=res[:])

            if os.environ.get("SKIP_E2") == "1":
                zres = sp.tile([P, 64], f32, tag="zres")
                nc.gpsimd.memset(zres[:], 0.0)
                for t in range(n_tiles):
                    nc.sync.dma_start(out=OUT[t * P:(t + 1) * P, :], in_=zres[:])
            else:
                for _erep in range(int(os.environ.get("KREP_E2", "1"))):
                    edge_phase(table2, 2, emit2)

    nc.compile()
    return nc


# ----------------------------------------------------------------------------
# Host entry point
# ----------------------------------------------------------------------------
def _make_inputs(inputs, meta, idx16, node_of_pos):
    N = meta["N"]; Npad = meta["Npad"]; n_cores = meta["n_cores"]
    x = np.asarray(inputs["x"], dtype=np.float32)
    xp = np.zeros((Npad, F_IN), dtype=np.float32)
    valid = node_of_pos >= 0
    xp[valid] = x[node_of_pos[valid]]
    xT = np.ascontiguousarray(xp.T).astype(bf16)

    W1 = np.asarray(inputs["W1"], dtype=np.float32)
    a_s1 = np.asarray(inputs["a_src1"], dtype=np.float32)
    a_d1 = np.asarray(inputs["a_dst1"], dtype=np.float32)
    A1 = np.zeros((64, 16), dtype=np.float32)
    for h in range(H1):
        A1[h * C1:(h + 1) * C1, h] = a_s1[h]
        A1[h * C1:(h + 1) * C1, 8 + h] = a_d1[h]
    W2 = np.asarray(inputs["W2"], dtype=np.float32)
    a_s2 = np.asarray(inputs["a_src2"], dtype=np.float32).reshape(C2, 1)
    a_d2 = np.asarray(inputs["a_dst2"], dtype=np.float32).reshape(C2, 1)
    A2 = np.concatenate([a_s2, a_d2], axis=1)
    common = dict(
        xT=xT, W1=W1, W1T=np.ascontiguousarray(W1.T), A1=A1,
        W2=W2, W2T=np.ascontiguousarray(W2.T), A2=A2,
        B1=np.asarray(inputs["b1"], np.float32).reshape(1, 64),
        B2=np.asarray(inputs["b2"], np.float32).reshape(1, C2),
    )
    return [dict(common, IDX=np.ascontiguousarray(idx16[k])) for k in range(n_cores)]


def kernel(**inputs):
    x = np.asarray(inputs["x"])
    edge_index = np.asarray(inputs["edge_index"])
    N = x.shape[0]
    meta, idx16, node_of_pos = host_prep(edge_index, N, NCORES)
    nc = build_kernel(meta)
    in_maps = _make_inputs(inputs, meta, idx16, node_of_pos)
    res = run_bass_kernel_spmd(nc, in_maps, list(range(NCORES)))
    out = np.empty((N, C2), dtype=np.float32)
    for k in range(NCORES):
        o = res.results[k]["OUT"]
        pos0 = k * meta["per_core"]
        nodes = node_of_pos[pos0:pos0 + meta["per_core"]]
        valid = nodes >= 0
        out[nodes[valid]] = o[valid.nonzero()[0]]
    return out

